# revision 28
# baseline (speedup 1.0000x reference)
"""Trainium2 Bass kernel for dense-transformer attention block.

Computes, for x [N, d] and weight [M, d] (N=M=8192, d=1024, fp32):
    scores = x @ W^T / sqrt(d)        # [N, M]
    probs  = softmax(scores, axis=-1)
    out    = probs @ W                # [N, d]

Sharding: rows of x (N) split across 8 NeuronCores; W replicated.

Per-core device algorithm (all matmuls bf16 with fp32 PSUM accumulation):
  - mm1 computes scores TRANSPOSED: sT[m_tile, n_block] = W @ x^T so that
    the softmax matmul (mm2) can consume exp(sT) directly as the stationary
    operand with W in natural [M, d] layout for the moving operand.
  - softmax denominators come from a ones column appended to W on the host
    (wA = [W | 1], 1025 cols). mm2 streams wA in 3 roughly equal chunks
    (342/342/341 <= 512-col PSUM bank limit); the denominator accumulates
    as the last column of the last chunk. No partition-axis reduction and
    no extra matmul for the denominator.
  - max-subtraction is skipped: scores/sqrt(d) ~ N(0,1), |s|<8, exp is safe
    in fp32.
  - final out = (u @ W) * (1/l) with the reciprocal applied per row after
    mm2, avoiding a pass over the [n, M] probability matrix.
  - the PE is pre-warmed with dummy matmuls at boot so the DVFS ramp
    (0.65 -> 2.4 GHz over ~3us of continuous busy) completes during the
    initial DMA fill instead of eating into real matmul throughput.
  - output is stored bf16 (host casts back to fp32): halves the store
    traffic and shortens the end-of-kernel drain; the added rounding is
    ~0.1% rel err against a 2e-2 budget.

Host side does the layout prep (transpose + bf16 cast + ones concat + row
sharding) and the gather/concat of per-core outputs.
"""

import os
from contextlib import ExitStack

import numpy as np
import ml_dtypes

import concourse.mybir as mybir
import concourse.tile as tile
from concourse import bacc
from concourse.bass import ts, ds
from concourse.bass_utils import run_bass_kernel_spmd

# bass_utils' trace path does a bare `from antenv.axon_hooks import ...`;
# some images ship antenv without that submodule, which would crash a run
# that forces BASS_TRACE. Provide a no-op hook registry in that case so
# bass_utils falls back to its own "hook isn't registered" path instead.
try:
    import antenv.axon_hooks  # noqa: F401
except ImportError:  # pragma: no cover
    import sys as _sys
    import types as _types

    _ah = _types.ModuleType("antenv.axon_hooks")
    _ah._hook = None
    _ah.set_axon_ntff_profile_hook = lambda h: setattr(_ah, "_hook", h)
    _ah.get_axon_ntff_profile_hook = lambda: _ah._hook
    try:
        import antenv as _antenv

        _antenv.axon_hooks = _ah
        _sys.modules["antenv.axon_hooks"] = _ah
    except ImportError:
        pass

# Problem shape (hardcoded per contract; spec nn_Model_39676907887569)
N_FULL = 8192
D = 1024
M = 8192
N_CORES = 8
N_LOC = N_FULL // N_CORES  # 1024 rows per core
SCALE = 1.0 / 32.0         # 1/sqrt(d)

BF16 = mybir.dt.bfloat16
F32 = mybir.dt.float32
NP_BF16 = ml_dtypes.bfloat16


def _chunk_cols(total, limit=512):
    """Split `total` columns into the fewest chunks all <= limit, near-equal."""
    n = (total + limit - 1) // limit
    base = total // n
    rem = total % n
    sizes = [base + (1 if i < rem else 0) for i in range(n)]
    offs = [sum(sizes[:i]) for i in range(n)]
    return list(zip(offs, sizes))


def build_nc(n_loc=N_LOC, d=D, m=M, nb_rows=256, scale=SCALE,
             warmup=34, defer_window=10, defer_depth=7):
    """Build the per-core Bass program (same NEFF for all cores)."""
    assert n_loc % nb_rows == 0 and nb_rows % 128 == 0
    assert d % 128 == 0 and m % 128 == 0
    d_tiles = d // 128
    m_tiles = m // 128
    n_blocks = n_loc // nb_rows
    n_chunks = nb_rows // 128
    # mm2 moving-operand chunks over [W | ones] = d+1 columns
    d_chunks = _chunk_cols(d + 1)

    nc = bacc.Bacc(
        "TRN2",
        target_bir_lowering=False,
        debug=False,
        enable_asserts=False,
        num_devices=1,
    )

    # xT pre-swizzled host-side to [128, d_tiles, n_loc] so the single
    # whole-tensor load moves 16KB-contiguous per-partition lines.
    xT_dram = nc.dram_tensor("xT", [128, d_tiles, n_loc], BF16,
                             kind="ExternalInput").ap()
    # wT pre-arranged host-side as [d_tiles, 128, m] so each slab DMA moves
    # 2KB-contiguous lines (the flat [d, m] view would stride 256B lines).
    wT_dram = nc.dram_tensor("wT", [d_tiles, 128, m], BF16,
                             kind="ExternalInput").ap()
    # Packed cold-start tensors: head_a (sync queue) = [wT cols 0:512 |
    # x block-0 cols 0:256]; head_b (scalar queue) = wT cols 512:1024. The
    # two queues transfer in parallel at boot, so the PE's first stationary
    # and moving operands (head_a) arrive ~4us earlier than one 2.5MB blob.
    head_dram = nc.dram_tensor("head", [128, d_tiles, 768], BF16,
                               kind="ExternalInput").ap()
    headb_dram = nc.dram_tensor("head_b", [128, d_tiles, 512], BF16,
                                kind="ExternalInput").ap()
    # wA rows padded host-side to a 64B-aligned stride (2112B) so the 2050B
    # streamed lines start burst-aligned in HBM.
    wa_stride = ((d + 1) * 2 + 63) // 64 * 64 // 2  # cols incl. pad
    wA_dram = nc.dram_tensor("wA", [m, wa_stride], BF16, kind="ExternalInput").ap()
    out_dram = nc.dram_tensor("out", [n_loc, d], BF16, kind="ExternalOutput").ap()

    # Partition-major DRAM view of wT for the one-shot slab-0 load
    wT_pv = wT_dram.rearrange("a p c -> p a c")           # [128, d_tiles, m]
    # wT slab granularity: m-slabs of this many columns per (d_tile) piece
    wt_slab = min(1024, m)
    wt_slabs = m // wt_slab

    with tile.TileContext(nc) as tc:
        with ExitStack() as ctx:
            singles = ctx.enter_context(tc.tile_pool(name="singles", bufs=1))
            w2_pool = ctx.enter_context(tc.tile_pool(name="w2", bufs=16))
            u_pool = ctx.enter_context(tc.tile_pool(name="u", bufs=defer_depth + 2))
            o_pool = ctx.enter_context(tc.tile_pool(name="o", bufs=2))
            r_pool = ctx.enter_context(tc.tile_pool(name="r", bufs=4))
            s_psum = ctx.enter_context(tc.tile_pool(name="s_ps", bufs=2, space="PSUM"))
            acc_psum = ctx.enter_context(tc.tile_pool(name="acc", bufs=1, space="PSUM"))

            # PE warm-up: back-to-back dummy matmuls on a memset tile keep the
            # PE continuously busy from boot so the DVFS ramp finishes while
            # the first real operands are still in flight on DMA.
            if warmup:
                # DVE memset seeds the warmup operand; DVE's engine boot is
                # the earliest writer available (measured faster than a
                # seed-DMA through the queue boot path).
                warm_sb = singles.tile([128, 256], BF16)
                nc.vector.memset(warm_sb, 0)
                # Shares the s_ps tag (slot group) so no extra PSUM banks.
                warm_ps = s_psum.tile([128, 256], F32, name="s_ps")
                for _ in range(warmup):
                    nc.tensor.matmul(
                        warm_ps, lhsT=warm_sb[:, ds(0, 128)], rhs=warm_sb,
                        start=True, stop=True,
                    )

            # Resident weights / activations
            wT_sb = singles.tile([128, d_tiles, m], BF16)
            xT_sb = singles.tile([128, d_tiles, n_loc], BF16)
            head_sb = singles.tile([128, d_tiles, 768], BF16)
            headb_sb = singles.tile([128, d_tiles, 512], BF16)

            # Cold-start loads split across both hardware queues. The full
            # 2MB xT (first needed at block 1) is issued later, inside the
            # loop, to keep boot bandwidth for the critical chain.
            nc.sync.dma_start(head_sb, head_dram)
            nc.scalar.dma_start(headb_sb, headb_dram)

            def mm1_ops(nb, mt, dt_):
                """(lhsT, rhs) for mm1, preferring the packed head tensor
                during block 0 so the PE isn't gated on the big loads."""
                if mt < 4:
                    lhsT = head_sb[:, dt_, ds(mt * 128, 128)]
                elif mt < 8:
                    lhsT = headb_sb[:, dt_, ds((mt - 4) * 128, 128)]
                else:
                    lhsT = wT_sb[:, dt_, ts(mt, 128)]
                if nb == 0:
                    rhs = head_sb[:, dt_, ds(512, nb_rows)]
                else:
                    rhs = xT_sb[:, dt_, ds(nb * nb_rows, nb_rows)]
                return lhsT, rhs

            for nb in range(n_blocks):
                # Per-n_chunk PSUM accumulators, live across the whole m loop
                acc = []
                for nch in range(n_chunks):
                    chunks = [
                        acc_psum.tile([128, sz], F32, tag=f"acc_{nch}_{ci}",
                                      name=f"acc_{nch}_{ci}")
                        for ci, (_, sz) in enumerate(d_chunks)
                    ]
                    acc.append(chunks)

                # Software pipeline (depth 2 steady-state): mm2 for m_tile t
                # is issued after mm1 for t+2 so the ACT exp latency is fully
                # hidden behind at least one full mm1+mm2 batch on the PE.
                pending = []  # [(uT, w2), ...] awaiting mm2, oldest first

                def issue_mm2(uT, w2, first, last):
                    # On the very last flush of the kernel, emit the
                    # denominator-carrying chunks first so the normalize/store
                    # drain (which chases in reverse order) starts as early as
                    # possible. Middle blocks keep forward order to match the
                    # next block's acc-bank reuse (WAR) order.
                    nch_order = range(n_chunks)
                    ci_order = list(enumerate(d_chunks))
                    if last and nb == n_blocks - 1:
                        nch_order = reversed(list(nch_order))
                        ci_order = ci_order[::-1]
                    for nch in nch_order:
                        lhsT = uT[:, ts(nch, 128)]
                        for ci, (off, sz) in ci_order:
                            nc.tensor.matmul(
                                acc[nch][ci],
                                lhsT=lhsT,
                                rhs=w2[:, ds(off, sz)],
                                start=first,
                                stop=last,
                            )

                mm2_issued = 0

                def flush_pending(limit):
                    nonlocal mm2_issued
                    while len(pending) > limit:
                        uT_w2 = pending.pop(0)
                        issue_mm2(
                            *uT_w2,
                            first=(mm2_issued == 0),
                            last=(mm2_issued == m_tiles - 1),
                        )
                        mm2_issued += 1

                for mt in range(m_tiles):
                    # Pace the one-time resident wT slab loads one piece per
                    # iteration, a full slab ahead of first use, interleaved
                    # with the streamed wA tiles in FIFO order. (Ahead of the
                    # w2 tile in the FIFO: during the mm1-only cold window the
                    # slab pieces are the binding arrivals.)
                    if nb == 0 and mt < (wt_slabs - 1) * d_tiles:
                        ms, dt_ = mt // d_tiles + 1, mt % d_tiles
                        nc.sync.dma_start(
                            wT_sb[:, dt_, ds(ms * wt_slab, wt_slab)],
                            wT_dram[dt_, :, ds(ms * wt_slab, wt_slab)],
                        )
                    # Deferred whole-xT load, once the cold-start chain has
                    # drained (first needed at block 1).
                    if nb == 0 and mt == 24:
                        nc.scalar.dma_start(xT_sb, xT_dram)
                    w2 = w2_pool.tile([128, d + 1], BF16)
                    nc.sync.dma_start(w2, wA_dram[ts(mt, 128), ds(0, d + 1)])

                    s_ps = s_psum.tile([128, nb_rows], F32)
                    for dt_ in range(d_tiles):
                        lhsT, rhs = mm1_ops(nb, mt, dt_)
                        nc.tensor.matmul(
                            s_ps,
                            lhsT=lhsT,
                            rhs=rhs,
                            start=(dt_ == 0),
                            stop=(dt_ == d_tiles - 1),
                        )
                    uT = u_pool.tile([128, nb_rows], BF16)
                    nc.scalar.activation(uT, s_ps, mybir.ActivationFunctionType.Exp,
                                         scale=scale)

                    pending.append((uT, w2))
                    # During block 0's DMA cold-start, defer mm2 deeper so
                    # the PE runs ahead on mm1 (resident-input) work instead
                    # of stalling on the not-yet-spun-up wA stream. At every
                    # block start keep a small extra cushion so the previous
                    # block's normalize (which must read the acc banks before
                    # mm2 reuses them) stays off the critical path.
                    if nb == 0 and mt < defer_window:
                        limit = defer_depth
                    elif mt < 6:
                        limit = 4
                    else:
                        limit = 2
                    flush_pending(limit)

                flush_pending(0)

                # Normalize and store this n_block. The softmax denominator is
                # the last column of the last accumulator chunk. Work is
                # emitted per (nch, ci) in an order chasing the reversed final
                # mm2 flush, with each nch's chunks split across DVE and ACT
                # and the store for a chunk range issued as soon as both
                # engines' halves are done, so the end-of-block drain pipes.
                lci = len(d_chunks) - 1
                l_off = d_chunks[lci][1] - 1
                last_block = nb == n_blocks - 1
                nch_order = (
                    list(reversed(range(n_chunks))) if last_block
                    else list(range(n_chunks))
                )
                ci_order = list(enumerate(d_chunks))
                if last_block:
                    ci_order = ci_order[::-1]
                rcps = [None] * n_chunks
                os_ = [None] * n_chunks
                for nch in nch_order:
                    rcp = r_pool.tile([128, 1], F32, name=f"rcp{nch}")
                    nc.vector.reciprocal(rcp, acc[nch][lci][:, ds(l_off, 1)])
                    rcps[nch] = rcp
                    os_[nch] = o_pool.tile([128, d], BF16, name="o")
                for nch in nch_order:
                    for ci, (off, sz) in ci_order:
                        out_sz = sz - 1 if ci == lci else sz
                        src = acc[nch][ci][:, ds(0, out_sz)]
                        dst = os_[nch][:, ds(off, out_sz)]
                        # Middle blocks: all muls on DVE, keeping ACT free for
                        # the next block's exp chain. Last block: split across
                        # DVE and ACT so the final drain halves.
                        if last_block and (nch + ci) % 2 == 1:
                            nc.scalar.mul(dst, src, rcps[nch])
                        else:
                            nc.vector.tensor_scalar_mul(dst, in0=src,
                                                        scalar1=rcps[nch])
                    row0 = nb * nb_rows + nch * 128
                    nc.scalar.dma_start(out_dram[ds(row0, 128), :], os_[nch])

    nc.compile()
    return nc


_NC_CACHE = {}


def _get_nc(key=(N_LOC, D, M)):
    if key not in _NC_CACHE:
        _NC_CACHE[key] = build_nc(*key)
    return _NC_CACHE[key]


def kernel(x: np.ndarray, weight: np.ndarray) -> np.ndarray:
    x = np.ascontiguousarray(np.asarray(x, dtype=np.float32))
    w = np.ascontiguousarray(np.asarray(weight, dtype=np.float32))
    assert x.shape == (N_FULL, D) and w.shape == (M, D)

    # Host-side layout prep (cheap vs device work): bf16 casts + transposes
    w_bf = w.astype(NP_BF16)
    wa_stride = ((D + 1) * 2 + 63) // 64 * 64 // 2
    wA = np.zeros((M, wa_stride), NP_BF16)                    # [M, d+1 padded]
    wA[:, :D] = w_bf
    wA[:, D] = NP_BF16(1.0)
    wT_bf = np.ascontiguousarray(w_bf.T).reshape(8, 128, M)   # [d_tiles, 128, M]
    xT_full = x.astype(NP_BF16).T                             # [d, N]

    in_maps = []
    for c in range(N_CORES):
        xT_c = xT_full[:, c * N_LOC:(c + 1) * N_LOC]          # [d, n_loc]
        # partition-major swizzle: [128, d_tiles, n_loc]
        xT_p = np.ascontiguousarray(
            xT_c.reshape(8, 128, N_LOC).transpose(1, 0, 2)
        )
        head = np.empty((128, 8, 768), NP_BF16)
        head[:, :, :512] = wT_bf[:, :, :512].transpose(1, 0, 2)
        head[:, :, 512:] = xT_p[:, :, :256]
        head_b = np.ascontiguousarray(
            wT_bf[:, :, 512:1024].transpose(1, 0, 2))
        in_maps.append({"xT": xT_p, "wT": wT_bf, "wA": wA,
                        "head": np.ascontiguousarray(head),
                        "head_b": head_b})

    nc = _get_nc()
    trace = bool(int(os.environ.get("KERNEL_TRACE", "0")))
    res = run_bass_kernel_spmd(
        nc,
        in_maps,
        core_ids=list(range(N_CORES)),
        trace=trace,
    )
    if trace and res.exec_time_ns is not None:
        print(f"HW exec time: {res.exec_time_ns} ns")
        kernel.last_results = res
    out = np.concatenate(
        [r["out"].astype(np.float32) for r in res.results], axis=0
    )
    return out


kernel.last_results = None


# revision 29
# speedup vs baseline: 1.0186x; 1.0186x over previous
"""Trainium2 Bass kernel for dense-transformer attention block.

Computes, for x [N, d] and weight [M, d] (N=M=8192, d=1024, fp32):
    scores = x @ W^T / sqrt(d)        # [N, M]
    probs  = softmax(scores, axis=-1)
    out    = probs @ W                # [N, d]

Sharding: rows of x (N) split across 8 NeuronCores; W replicated.

Per-core device algorithm (all matmuls bf16 with fp32 PSUM accumulation):
  - mm1 computes scores TRANSPOSED: sT[m_tile, n_block] = W @ x^T so that
    the softmax matmul (mm2) can consume exp(sT) directly as the stationary
    operand with W in natural [M, d] layout for the moving operand.
  - softmax denominators come from a ones column appended to W on the host
    (wA = [W | 1], 1025 cols). mm2 streams wA in 3 roughly equal chunks
    (342/342/341 <= 512-col PSUM bank limit); the denominator accumulates
    as the last column of the last chunk. No partition-axis reduction and
    no extra matmul for the denominator.
  - max-subtraction is skipped: scores/sqrt(d) ~ N(0,1), |s|<8, exp is safe
    in fp32.
  - final out = (u @ W) * (1/l) with the reciprocal applied per row after
    mm2, avoiding a pass over the [n, M] probability matrix.
  - the PE is pre-warmed with dummy matmuls at boot so the DVFS ramp
    (0.65 -> 2.4 GHz over ~3us of continuous busy) completes during the
    initial DMA fill instead of eating into real matmul throughput.
  - output is stored bf16 (host casts back to fp32): halves the store
    traffic and shortens the end-of-kernel drain; the added rounding is
    ~0.1% rel err against a 2e-2 budget.

Host side does the layout prep (transpose + bf16 cast + ones concat + row
sharding) and the gather/concat of per-core outputs.
"""

import os
from contextlib import ExitStack

import numpy as np
import ml_dtypes

import concourse.mybir as mybir
import concourse.tile as tile
from concourse import bacc
from concourse.bass import ts, ds
from concourse.bass_utils import run_bass_kernel_spmd

# bass_utils' trace path does a bare `from antenv.axon_hooks import ...`;
# some images ship antenv without that submodule, which would crash a run
# that forces BASS_TRACE. Provide a no-op hook registry in that case so
# bass_utils falls back to its own "hook isn't registered" path instead.
try:
    import antenv.axon_hooks  # noqa: F401
except ImportError:  # pragma: no cover
    import sys as _sys
    import types as _types

    _ah = _types.ModuleType("antenv.axon_hooks")
    _ah._hook = None
    _ah.set_axon_ntff_profile_hook = lambda h: setattr(_ah, "_hook", h)
    _ah.get_axon_ntff_profile_hook = lambda: _ah._hook
    try:
        import antenv as _antenv

        _antenv.axon_hooks = _ah
        _sys.modules["antenv.axon_hooks"] = _ah
    except ImportError:
        pass

# Problem shape (hardcoded per contract; spec nn_Model_39676907887569)
N_FULL = 8192
D = 1024
M = 8192
N_CORES = 8
N_LOC = N_FULL // N_CORES  # 1024 rows per core
SCALE = 1.0 / 32.0         # 1/sqrt(d)

BF16 = mybir.dt.bfloat16
F32 = mybir.dt.float32
NP_BF16 = ml_dtypes.bfloat16


def _chunk_cols(total, limit=512):
    """Split `total` columns into the fewest chunks all <= limit, near-equal."""
    n = (total + limit - 1) // limit
    base = total // n
    rem = total % n
    sizes = [base + (1 if i < rem else 0) for i in range(n)]
    offs = [sum(sizes[:i]) for i in range(n)]
    return list(zip(offs, sizes))


def build_nc(n_loc=N_LOC, d=D, m=M, nb_rows=256, scale=SCALE,
             warmup=34, defer_window=10, defer_depth=7):
    """Build the per-core Bass program (same NEFF for all cores)."""
    assert n_loc % nb_rows == 0 and nb_rows % 128 == 0
    assert d % 128 == 0 and m % 128 == 0
    d_tiles = d // 128
    m_tiles = m // 128
    n_blocks = n_loc // nb_rows
    n_chunks = nb_rows // 128
    # mm2 moving-operand chunks over [W | ones] = d+1 columns
    d_chunks = _chunk_cols(d + 1)

    nc = bacc.Bacc(
        "TRN2",
        target_bir_lowering=False,
        debug=False,
        enable_asserts=False,
        num_devices=1,
    )

    # xT pre-swizzled host-side to [128, d_tiles, n_loc] so the single
    # whole-tensor load moves 16KB-contiguous per-partition lines.
    xT_dram = nc.dram_tensor("xT", [128, d_tiles, n_loc], BF16,
                             kind="ExternalInput").ap()
    # wT pre-arranged host-side as [d_tiles, 128, m] so each slab DMA moves
    # 2KB-contiguous lines (the flat [d, m] view would stride 256B lines).
    wT_dram = nc.dram_tensor("wT", [d_tiles, 128, m], BF16,
                             kind="ExternalInput").ap()
    # Packed cold-start tensor: per (partition, d_tile) 1024 contiguous cols
    # = [wT cols 0:768 | x block-0 cols 0:256], one 2MB DMA with 16KB lines
    # covering the first six m_tiles' stationaries and all of block 0's
    # moving operand.
    head_dram = nc.dram_tensor("head", [128, d_tiles, 1024], BF16,
                               kind="ExternalInput").ap()
    # wA rows padded host-side to a 64B-aligned stride (2112B) so the 2050B
    # streamed lines start burst-aligned in HBM.
    wa_stride = ((d + 1) * 2 + 63) // 64 * 64 // 2  # cols incl. pad
    wA_dram = nc.dram_tensor("wA", [m, wa_stride], BF16, kind="ExternalInput").ap()
    out_dram = nc.dram_tensor("out", [n_loc, d], BF16, kind="ExternalOutput").ap()

    # Partition-major DRAM view of wT for the one-shot slab-0 load
    wT_pv = wT_dram.rearrange("a p c -> p a c")           # [128, d_tiles, m]
    # wT slab granularity: m-slabs of this many columns per (d_tile) piece
    wt_slab = min(1024, m)
    wt_slabs = m // wt_slab

    with tile.TileContext(nc) as tc:
        with ExitStack() as ctx:
            singles = ctx.enter_context(tc.tile_pool(name="singles", bufs=1))
            w2_pool = ctx.enter_context(tc.tile_pool(name="w2", bufs=16))
            u_pool = ctx.enter_context(tc.tile_pool(name="u", bufs=defer_depth + 2))
            o_pool = ctx.enter_context(tc.tile_pool(name="o", bufs=2))
            r_pool = ctx.enter_context(tc.tile_pool(name="r", bufs=4))
            s_psum = ctx.enter_context(tc.tile_pool(name="s_ps", bufs=2, space="PSUM"))
            acc_psum = ctx.enter_context(tc.tile_pool(name="acc", bufs=1, space="PSUM"))

            # PE warm-up: back-to-back dummy matmuls on a memset tile keep the
            # PE continuously busy from boot so the DVFS ramp finishes while
            # the first real operands are still in flight on DMA.
            if warmup:
                # DVE memset seeds the warmup operand; DVE's engine boot is
                # the earliest writer available (measured faster than a
                # seed-DMA through the queue boot path).
                warm_sb = singles.tile([128, 256], BF16)
                nc.vector.memset(warm_sb, 0)
                # Shares the s_ps tag (slot group) so no extra PSUM banks.
                warm_ps = s_psum.tile([128, 256], F32, name="s_ps")
                for _ in range(warmup):
                    nc.tensor.matmul(
                        warm_ps, lhsT=warm_sb[:, ds(0, 128)], rhs=warm_sb,
                        start=True, stop=True,
                    )

            # Resident weights / activations
            wT_sb = singles.tile([128, d_tiles, m], BF16)
            xT_sb = singles.tile([128, d_tiles, n_loc], BF16)
            head_sb = singles.tile([128, d_tiles, 1024], BF16)

            # Cold-start loads, strictly ordered on the sync queue: head
            # first, then the tail of slab 0. The full 2MB xT (first needed
            # at block 1) is issued later, inside the loop, to keep boot
            # bandwidth for the critical chain.
            nc.sync.dma_start(head_sb, head_dram)
            nc.sync.dma_start(
                wT_sb[:, :, ds(768, wt_slab - 768)],
                wT_pv[:, :, ds(768, wt_slab - 768)],
            )

            def mm1_ops(nb, mt, dt_):
                """(lhsT, rhs) for mm1, preferring the packed head tensor
                during block 0 so the PE isn't gated on the big loads."""
                if mt < 6:
                    lhsT = head_sb[:, dt_, ds(mt * 128, 128)]
                else:
                    lhsT = wT_sb[:, dt_, ts(mt, 128)]
                if nb == 0:
                    rhs = head_sb[:, dt_, ds(768, nb_rows)]
                else:
                    rhs = xT_sb[:, dt_, ds(nb * nb_rows, nb_rows)]
                return lhsT, rhs

            for nb in range(n_blocks):
                # Per-n_chunk PSUM accumulators, live across the whole m loop
                acc = []
                for nch in range(n_chunks):
                    chunks = [
                        acc_psum.tile([128, sz], F32, tag=f"acc_{nch}_{ci}",
                                      name=f"acc_{nch}_{ci}")
                        for ci, (_, sz) in enumerate(d_chunks)
                    ]
                    acc.append(chunks)

                # Software pipeline (depth 2 steady-state): mm2 for m_tile t
                # is issued after mm1 for t+2 so the ACT exp latency is fully
                # hidden behind at least one full mm1+mm2 batch on the PE.
                pending = []  # [(uT, w2), ...] awaiting mm2, oldest first

                def issue_mm2(uT, w2, first, last):
                    # On the very last flush of the kernel, emit the
                    # denominator-carrying chunks first so the normalize/store
                    # drain (which chases in reverse order) starts as early as
                    # possible. Middle blocks keep forward order to match the
                    # next block's acc-bank reuse (WAR) order.
                    nch_order = range(n_chunks)
                    ci_order = list(enumerate(d_chunks))
                    if last and nb == n_blocks - 1:
                        nch_order = reversed(list(nch_order))
                        ci_order = ci_order[::-1]
                    for nch in nch_order:
                        lhsT = uT[:, ts(nch, 128)]
                        for ci, (off, sz) in ci_order:
                            nc.tensor.matmul(
                                acc[nch][ci],
                                lhsT=lhsT,
                                rhs=w2[:, ds(off, sz)],
                                start=first,
                                stop=last,
                            )

                mm2_issued = 0

                def flush_pending(limit):
                    nonlocal mm2_issued
                    while len(pending) > limit:
                        uT_w2 = pending.pop(0)
                        issue_mm2(
                            *uT_w2,
                            first=(mm2_issued == 0),
                            last=(mm2_issued == m_tiles - 1),
                        )
                        mm2_issued += 1

                for mt in range(m_tiles):
                    # Pace the one-time resident wT slab loads one piece per
                    # iteration, a full slab ahead of first use, interleaved
                    # with the streamed wA tiles in FIFO order.
                    if nb == 0 and mt < (wt_slabs - 1) * d_tiles:
                        ms, dt_ = mt // d_tiles + 1, mt % d_tiles
                        nc.sync.dma_start(
                            wT_sb[:, dt_, ds(ms * wt_slab, wt_slab)],
                            wT_dram[dt_, :, ds(ms * wt_slab, wt_slab)],
                        )
                    # Deferred whole-xT load, once the cold-start chain has
                    # drained (first needed at block 1).
                    if nb == 0 and mt == 24:
                        nc.scalar.dma_start(xT_sb, xT_dram)
                    w2 = w2_pool.tile([128, d + 1], BF16)
                    nc.sync.dma_start(w2, wA_dram[ts(mt, 128), ds(0, d + 1)])

                    s_ps = s_psum.tile([128, nb_rows], F32)
                    for dt_ in range(d_tiles):
                        lhsT, rhs = mm1_ops(nb, mt, dt_)
                        nc.tensor.matmul(
                            s_ps,
                            lhsT=lhsT,
                            rhs=rhs,
                            start=(dt_ == 0),
                            stop=(dt_ == d_tiles - 1),
                        )
                    uT = u_pool.tile([128, nb_rows], BF16)
                    nc.scalar.activation(uT, s_ps, mybir.ActivationFunctionType.Exp,
                                         scale=scale)

                    pending.append((uT, w2))
                    # During block 0's DMA cold-start, defer mm2 deeper so
                    # the PE runs ahead on mm1 (resident-input) work instead
                    # of stalling on the not-yet-spun-up wA stream. At every
                    # block start keep a small extra cushion so the previous
                    # block's normalize (which must read the acc banks before
                    # mm2 reuses them) stays off the critical path.
                    if nb == 0 and mt < defer_window:
                        limit = defer_depth
                    elif mt < 6:
                        limit = 4
                    else:
                        limit = 2
                    flush_pending(limit)

                flush_pending(0)

                # Normalize and store this n_block. The softmax denominator is
                # the last column of the last accumulator chunk. Work is
                # emitted per (nch, ci) in an order chasing the reversed final
                # mm2 flush, with each nch's chunks split across DVE and ACT
                # and the store for a chunk range issued as soon as both
                # engines' halves are done, so the end-of-block drain pipes.
                lci = len(d_chunks) - 1
                l_off = d_chunks[lci][1] - 1
                last_block = nb == n_blocks - 1
                nch_order = (
                    list(reversed(range(n_chunks))) if last_block
                    else list(range(n_chunks))
                )
                ci_order = list(enumerate(d_chunks))
                if last_block:
                    ci_order = ci_order[::-1]
                rcps = [None] * n_chunks
                os_ = [None] * n_chunks
                for nch in nch_order:
                    rcp = r_pool.tile([128, 1], F32, name=f"rcp{nch}")
                    nc.vector.reciprocal(rcp, acc[nch][lci][:, ds(l_off, 1)])
                    rcps[nch] = rcp
                    os_[nch] = o_pool.tile([128, d], BF16, name="o")
                for nch in nch_order:
                    for ci, (off, sz) in ci_order:
                        out_sz = sz - 1 if ci == lci else sz
                        src = acc[nch][ci][:, ds(0, out_sz)]
                        dst = os_[nch][:, ds(off, out_sz)]
                        # Middle blocks: all muls on DVE, keeping ACT free for
                        # the next block's exp chain. Last block: split across
                        # DVE and ACT so the final drain halves.
                        if last_block and (nch + ci) % 2 == 1:
                            nc.scalar.mul(dst, src, rcps[nch])
                        else:
                            nc.vector.tensor_scalar_mul(dst, in0=src,
                                                        scalar1=rcps[nch])
                    row0 = nb * nb_rows + nch * 128
                    nc.scalar.dma_start(out_dram[ds(row0, 128), :], os_[nch])

    nc.compile()
    return nc


_NC_CACHE = {}


def _get_nc(key=(N_LOC, D, M)):
    if key not in _NC_CACHE:
        _NC_CACHE[key] = build_nc(*key)
    return _NC_CACHE[key]


def kernel(x: np.ndarray, weight: np.ndarray) -> np.ndarray:
    x = np.ascontiguousarray(np.asarray(x, dtype=np.float32))
    w = np.ascontiguousarray(np.asarray(weight, dtype=np.float32))
    assert x.shape == (N_FULL, D) and w.shape == (M, D)

    # Host-side layout prep (cheap vs device work): bf16 casts + transposes
    w_bf = w.astype(NP_BF16)
    wa_stride = ((D + 1) * 2 + 63) // 64 * 64 // 2
    wA = np.zeros((M, wa_stride), NP_BF16)                    # [M, d+1 padded]
    wA[:, :D] = w_bf
    wA[:, D] = NP_BF16(1.0)
    wT_bf = np.ascontiguousarray(w_bf.T).reshape(8, 128, M)   # [d_tiles, 128, M]
    xT_full = x.astype(NP_BF16).T                             # [d, N]

    in_maps = []
    for c in range(N_CORES):
        xT_c = xT_full[:, c * N_LOC:(c + 1) * N_LOC]          # [d, n_loc]
        # partition-major swizzle: [128, d_tiles, n_loc]
        xT_p = np.ascontiguousarray(
            xT_c.reshape(8, 128, N_LOC).transpose(1, 0, 2)
        )
        head = np.empty((128, 8, 1024), NP_BF16)
        head[:, :, :768] = wT_bf[:, :, :768].transpose(1, 0, 2)
        head[:, :, 768:] = xT_p[:, :, :256]
        in_maps.append({"xT": xT_p, "wT": wT_bf, "wA": wA,
                        "head": np.ascontiguousarray(head)})

    nc = _get_nc()
    trace = bool(int(os.environ.get("KERNEL_TRACE", "0")))
    res = run_bass_kernel_spmd(
        nc,
        in_maps,
        core_ids=list(range(N_CORES)),
        trace=trace,
    )
    if trace and res.exec_time_ns is not None:
        print(f"HW exec time: {res.exec_time_ns} ns")
        kernel.last_results = res
    out = np.concatenate(
        [r["out"].astype(np.float32) for r in res.results], axis=0
    )
    return out


kernel.last_results = None
